# revision 17
# baseline (speedup 1.0000x reference)
"""Trainium2 Bass kernel for nn_GapDecoder.

Computes gaps[i,j] = proj[i] + proj[j] + b2 where
proj = relu(x @ W1 + b1) @ w2, x: [8192, 512] f32.

Strategy (8 NeuronCores, block-partitioned, collective-free):
  The [8192, 8192] output is an 8x8 grid of [1024, 1024] blocks. Core m
  handles chunk set Lm = {m, m+1, m+2, m+4} (mod 8); one block per
  difference delta = Lm[q]-Lm[p] (mod 8) makes the union over cores an
  exact partition of all 64 blocks. With the column-proj buffer laid
  out in position order (0,3,1,2), the core's 8 blocks group into five
  contiguous bands whose dependencies arrive in order under compute
  order (0,3,1,2) — store-ready bytes come in smooth 2/2/4/8MB waves.

  All HBM traffic and matmul operands are bf16 (rel-err budget 2e-2;
  bf16 costs ~0.5%): 4MB x read + 16MB output write per core. Reads
  are dispatched on the sync engine's HW-DGE queue and stores on the
  activation engine's queue, so store transfers overlap the read tail
  instead of queueing behind it. All outer-sum adds run on the DVE in
  its 16-bit 2x mode (~660GB/s of bf16 tiles, above the ~420GB/s DMA
  drain rate); the scalar engine only dispatches stores. Output is
  upcast to f32 on host.
"""

import sys

sys.path.insert(0, "/opt/trn_rl_repo")

import ml_dtypes
import numpy as np

N, D, H = 8192, 512, 32
NCORES = 8
CHUNK = 1024  # block edge / proj chunk
NLOC = 4  # chunks per core
LROWS = NLOC * CHUNK  # local rows per core
STRIPE = 512  # rows per PE stripe
NSTRIP = LROWS // STRIPE
KCH = D // 128
HALF = CHUNK // STRIPE  # stripes per chunk

# local chunk offsets (positions 0..3 hold offsets 0,1,2,4)
LOCAL_OFFS = (0, 1, 2, 4)
# column-proj buffer holds chunk positions in this order; paired with
# COMPUTE_ORDER below it makes every band a contiguous slice that
# becomes ready as early as possible
COL_ORDER = (0, 3, 1, 2)
COL_BASE = {0: 0, 3: CHUNK, 1: 2 * CHUNK, 2: 3 * CHUNK}
COMPUTE_ORDER = (0, 3, 1, 2)
# emission stages: (row position, sbuf col offset, width, dram col offset)
BANDS = (
    (0, 0, CHUNK, 0),  # diag block (0,0): needs {0}
    (0, CHUNK, CHUNK, CHUNK),  # block (0,3): needs {0,3}
    (1, 0, 2 * CHUNK, 2 * CHUNK),  # band1 (1,{3,0}): needs {1,3,0}
    (0, 2 * CHUNK, 2 * CHUNK, 4 * CHUNK),  # band0R (0,{1,2}): needs {0,1,2}
    (3, 2 * CHUNK, 2 * CHUNK, 6 * CHUNK),  # band3 (3,{1,2}): needs {3,1,2}
)

# const blob layout (free offsets in the [128, *] tiles)
CB_W1 = 0  # bf16 [128, KCH*H]   w1, p-major
CB_W2B = KCH * H  # bf16 [32, 128]  w2 replicated (rows 0..31)
CB_W2 = KCH * H + 128  # bf16 [32, 1]
CB_BF_W = KCH * H + 129
CF_B1 = 0  # f32 [32, 1]
CF_B2 = 1  # f32 [128, 1]
CF_W = 2

_state = {}

# Set by run for test harnesses that want profile info (see test.py).
LAST_RESULTS = None


def _build():
    from concourse import bacc, tile, mybir

    f32 = mybir.dt.float32
    bf16 = mybir.dt.bfloat16
    nc = bacc.Bacc(
        "TRN2", target_bir_lowering=False, debug=False, num_devices=NCORES
    )

    # partition-major pack: chunk L at rows [L*128, (L+1)*128), row p =
    # partition p holding [stripe, k, 512] contiguously (8KB/partition)
    xT_d = nc.dram_tensor(
        "xT4", [NLOC * 128, HALF * KCH * STRIPE], bf16, kind="ExternalInput"
    )
    cb_d = nc.dram_tensor("cb", [128, CB_BF_W], bf16, kind="ExternalInput")
    cf_d = nc.dram_tensor("cf", [128, CF_W], f32, kind="ExternalInput")
    # five stage regions side by side: [1024, 1+1+2+2+2 chunks] bf16
    out_d = nc.dram_tensor("out", [CHUNK, 8 * CHUNK], bf16, kind="ExternalOutput")

    with tile.TileContext(nc) as tc:
        with (
            tc.tile_pool(name="const", bufs=1) as cpool,
            tc.tile_pool(name="xkp", bufs=4) as xkpool,
            tc.tile_pool(name="work", bufs=4) as wpool,
            tc.tile_pool(name="big", bufs=12) as bigpool,
            tc.tile_pool(name="psum", bufs=4, space="PSUM") as pspool,
            tc.tile_pool(name="pspc", bufs=2, space="PSUM") as pspc,
            tc.tile_pool(name="psbc", bufs=2, space="PSUM") as psbc,
        ):
            # ---- constants: two packed blobs on the ACT queue, so the
            # sync queue's first dispatch is the first x read ----
            cb_sb = cpool.tile([128, CB_BF_W], bf16)
            nc.scalar.dma_start(cb_sb[:], cb_d.ap())
            cf_sb = cpool.tile([128, CF_W], f32)
            nc.scalar.dma_start(cf_sb[:], cf_d.ap())

            w1_sb = cb_sb[:, CB_W1 : CB_W1 + KCH * H].rearrange(
                "p (k h) -> p k h", k=KCH
            )
            w2b_sb = cb_sb[0:H, CB_W2B : CB_W2B + 128]
            w2_sb = cb_sb[0:H, CB_W2 : CB_W2 + 1]
            b1_sb = cf_sb[0:H, CF_B1 : CF_B1 + 1]
            b2b_sb = cf_sb[:, CF_B2 : CF_B2 + 1]

            # column proj (+b2) for all 4 chunks, position order COL_ORDER
            bcolall = cpool.tile([128, NLOC * CHUNK], bf16)
            # per-partition proj scalars ([128, CHUNK//128] per local chunk)
            # (must stay f32: tensor_scalar requires an f32 scalar operand)
            projcol = [
                cpool.tile([128, CHUNK // 128], f32, name=f"projcol{i}")
                for i in range(NLOC)
            ]

            xks = {}
            for loc in COMPUTE_ORDER:
                xk = xkpool.tile(
                    [128, HALF, KCH * STRIPE], bf16, tag="xk", name=f"xk{loc}"
                )
                nc.sync.dma_start(
                    xk[:],
                    xT_d.ap()[loc * 128 : (loc + 1) * 128, :].rearrange(
                        "p (s k) -> p s k", s=HALF
                    ),
                )
                xks[loc] = xk

            emit_ctr = [0]

            def emit_band(p, c0, w, d0, groups=range(CHUNK // 128)):
                # adds all on DVE (16-bit 2x mode); store dispatches
                # alternate between the two HW-DGE queues (sync + ACT) so
                # descriptor generation is not the store-phase bottleneck
                for g in groups:
                    ot = bigpool.tile(
                        [128, w], bf16, tag="ot", name=f"ot{d0}_{g}"
                    )
                    nc.vector.tensor_scalar_add(
                        ot[:], bcolall[:, c0 : c0 + w], projcol[p][:, g : g + 1]
                    )
                    r0 = g * 128
                    # first tiles go on the ACT queue only: the sync queue
                    # still has pending x reads in front of it
                    k = emit_ctr[0]
                    emit_ctr[0] += 1
                    eng = nc.scalar if (k < 8 or k % 2 == 0) else nc.sync
                    eng.dma_start(out_d.ap()[r0 : r0 + 128, d0 : d0 + w], ot[:])

            done = set()
            emitted = set()
            for loc in COMPUTE_ORDER:
                xk = xks[loc]
                pc_ps = None
                if loc != 2:
                    # batched proj scalars: 4 one-column matmuls per stripe
                    # into one PSUM tile, one copy per stripe
                    pc_ps = pspc.tile([128, CHUNK // 128], f32, tag="pc")
                for half in range(HALF):
                    seqT_ps = pspool.tile([H, STRIPE], f32, tag="seqT")
                    for k in range(KCH):
                        nc.tensor.matmul(
                            seqT_ps[:],
                            w1_sb[:, k, :],
                            xk[:, half, k * STRIPE : (k + 1) * STRIPE],
                            start=(k == 0),
                            stop=(k == KCH - 1),
                        )
                    seqT_sb = wpool.tile([H, STRIPE], bf16, tag="seqT_sb")
                    # relu(x + b1) as a fused DVE op
                    nc.vector.tensor_scalar(
                        seqT_sb[:],
                        seqT_ps[:],
                        b1_sb,
                        0.0,
                        op0=mybir.AluOpType.add,
                        op1=mybir.AluOpType.max,
                    )
                    # broadcast proj of this stripe across all 128 partitions
                    # in one matmul, folding b2 into the psum->sbuf copy
                    bc_ps = psbc.tile([128, STRIPE], f32, tag="bc")
                    nc.tensor.matmul(bc_ps[:], w2b_sb, seqT_sb[:])
                    base = COL_BASE[loc]
                    nc.vector.tensor_scalar_add(
                        bcolall[:, base + half * STRIPE : base + (half + 1) * STRIPE],
                        bc_ps[:],
                        b2b_sb,
                    )
                    # position 2 never appears as a band row; skip its scalars
                    if loc != 2:
                        npc = STRIPE // 128
                        for c in range(npc):
                            col = half * npc + c
                            nc.tensor.matmul(
                                pc_ps[:, col : col + 1],
                                seqT_sb[:, c * 128 : (c + 1) * 128],
                                w2_sb,
                            )
                        nc.vector.tensor_copy(
                            projcol[loc][:, half * npc : (half + 1) * npc],
                            pc_ps[:, half * npc : (half + 1) * npc],
                        )
                    if loc == COMPUTE_ORDER[0] and half == 0:
                        # the top-left [512,512] of the diag block depends
                        # only on this first stripe: emit it now so the
                        # first stores dispatch ~4us earlier
                        emit_band(0, 0, STRIPE, 0, groups=range(4))

                done.add(loc)
                for bi, (p, c0, w, d0) in enumerate(BANDS):
                    need = {p} | {
                        q for q in range(NLOC) if c0 <= COL_BASE[q] < c0 + w
                    }
                    if bi not in emitted and need <= done:
                        emitted.add(bi)
                        if bi == 0:
                            # rest of the diag block around the early piece
                            emit_band(0, STRIPE, STRIPE, STRIPE, groups=range(4))
                            emit_band(0, 0, CHUNK, 0, groups=range(4, 8))
                        else:
                            emit_band(p, c0, w, d0)

            for bi, (p, c0, w, d0) in enumerate(BANDS):
                if bi not in emitted:
                    emitted.add(bi)
                    emit_band(p, c0, w, d0)

    nc.compile()
    return nc


def kernel(gathered_sequences, W1, b1, w2, b2):
    global LAST_RESULTS
    from concourse import bass_utils

    if "nc" not in _state:
        _state["nc"] = _build()
    nc = _state["nc"]

    bf = ml_dtypes.bfloat16
    x = np.ascontiguousarray(gathered_sequences, dtype=np.float32)
    xT = np.ascontiguousarray(x.T)  # [D, N]

    # const blobs
    cb = np.zeros((128, CB_BF_W), dtype=bf)
    W1b = np.asarray(W1, dtype=np.float32).astype(bf)  # [D, H]
    # w1 field: [p, k*H + h] = W1[k*128 + p, h]
    cb[:, CB_W1 : CB_W1 + KCH * H] = (
        W1b.reshape(KCH, 128, H).transpose(1, 0, 2).reshape(128, KCH * H)
    )
    w2c = np.reshape(w2, (H, 1)).astype(np.float32).astype(bf)
    cb[0:H, CB_W2B : CB_W2B + 128] = np.repeat(w2c, 128, axis=1)
    cb[0:H, CB_W2 : CB_W2 + 1] = w2c
    cf = np.zeros((128, CF_W), dtype=np.float32)
    cf[0:H, CF_B1] = np.reshape(b1, (H,)).astype(np.float32)
    cf[:, CF_B2] = float(np.reshape(b2, ()))

    in_maps = []
    for m in range(NCORES):
        locs = [(m + a) % NCORES for a in LOCAL_OFFS]
        xT4 = np.concatenate(
            [xT[:, L * CHUNK : (L + 1) * CHUNK] for L in locs], axis=1
        )  # [D, LROWS]
        # partition-major pack: [NLOC*128, HALF*KCH*STRIPE] where row
        # L*128+p holds chunk L's [stripe, k, 512] block for partition p
        xT4p = np.ascontiguousarray(
            xT4.reshape(KCH, 128, NLOC, HALF, STRIPE)
            .transpose(2, 1, 3, 0, 4)
            .reshape(NLOC * 128, HALF * KCH * STRIPE)
            .astype(bf)
        )
        in_maps.append({"xT4": xT4p, "cb": cb, "cf": cf})

    res = bass_utils.run_bass_kernel_spmd(nc, in_maps, core_ids=list(range(NCORES)))
    LAST_RESULTS = res

    out = np.empty((N, N), dtype=np.float32)
    for m in range(NCORES):
        locs = [(m + a) % NCORES for a in LOCAL_OFFS]
        bands = np.asarray(res.results[m]["out"]).astype(np.float32)
        for p, c0, w, d0 in BANDS:
            gr = locs[p]
            cols = [
                COL_ORDER[i] for i in range(NLOC) if c0 <= i * CHUNK < c0 + w
            ]
            for j, q in enumerate(cols):
                gc = locs[q]
                out[
                    gr * CHUNK : (gr + 1) * CHUNK, gc * CHUNK : (gc + 1) * CHUNK
                ] = bands[:, d0 + j * CHUNK : d0 + (j + 1) * CHUNK]
    return out


# revision 18
# speedup vs baseline: 1.0225x; 1.0225x over previous
"""Trainium2 Bass kernel for nn_GapDecoder.

Computes gaps[i,j] = proj[i] + proj[j] + b2 where
proj = relu(x @ W1 + b1) @ w2, x: [8192, 512] f32.

Strategy (8 NeuronCores, block-partitioned, collective-free):
  The [8192, 8192] output is an 8x8 grid of [1024, 1024] blocks. Core m
  handles chunk set Lm = {m, m+1, m+2, m+4} (mod 8); one block per
  difference delta = Lm[q]-Lm[p] (mod 8) makes the union over cores an
  exact partition of all 64 blocks. With the column-proj buffer laid
  out in position order (0,3,1,2), the core's 8 blocks group into five
  contiguous bands whose dependencies arrive in order under compute
  order (0,3,1,2) — store-ready bytes come in smooth 2/2/4/8MB waves.

  All HBM traffic and matmul operands are bf16 (rel-err budget 2e-2;
  bf16 costs ~0.5%): 4MB x read + 16MB output write per core. Reads
  are dispatched on the sync engine's HW-DGE queue and stores on the
  activation engine's queue, so store transfers overlap the read tail
  instead of queueing behind it. All outer-sum adds run on the DVE in
  its 16-bit 2x mode (~660GB/s of bf16 tiles, above the ~420GB/s DMA
  drain rate); the scalar engine only dispatches stores. Output is
  upcast to f32 on host.
"""

import sys

sys.path.insert(0, "/opt/trn_rl_repo")

import ml_dtypes
import numpy as np

N, D, H = 8192, 512, 32
NCORES = 8
CHUNK = 1024  # block edge / proj chunk
NLOC = 4  # chunks per core
LROWS = NLOC * CHUNK  # local rows per core
STRIPE = 512  # rows per PE stripe
NSTRIP = LROWS // STRIPE
KCH = D // 128
HALF = CHUNK // STRIPE  # stripes per chunk

# local chunk offsets (positions 0..3 hold offsets 0,1,2,4)
LOCAL_OFFS = (0, 1, 2, 4)
# column-proj buffer holds chunk positions in this order; paired with
# COMPUTE_ORDER below it makes every band a contiguous slice that
# becomes ready as early as possible
COL_ORDER = (0, 3, 1, 2)
COL_BASE = {0: 0, 3: CHUNK, 1: 2 * CHUNK, 2: 3 * CHUNK}
COMPUTE_ORDER = (0, 3, 1, 2)
# emission stages: (row position, sbuf col offset, width, dram col offset)
BANDS = (
    (0, 0, CHUNK, 0),  # diag block (0,0): needs {0}
    (0, CHUNK, CHUNK, CHUNK),  # block (0,3): needs {0,3}
    (1, 0, 2 * CHUNK, 2 * CHUNK),  # band1 (1,{3,0}): needs {1,3,0}
    (0, 2 * CHUNK, 2 * CHUNK, 4 * CHUNK),  # band0R (0,{1,2}): needs {0,1,2}
    (3, 2 * CHUNK, 2 * CHUNK, 6 * CHUNK),  # band3 (3,{1,2}): needs {3,1,2}
)

# const blob layout (free offsets in the [128, *] tiles)
CB_W1 = 0  # bf16 [128, KCH*H]   w1, p-major
CB_W2B = KCH * H  # bf16 [32, 128]  w2 replicated (rows 0..31)
CB_W2 = KCH * H + 128  # bf16 [32, 1]
CB_BF_W = KCH * H + 129
CF_B1 = 0  # f32 [32, 1]
CF_B2 = 1  # f32 [128, 1]
CF_W = 2

_state = {}

# Set by run for test harnesses that want profile info (see test.py).
LAST_RESULTS = None


def _build():
    from concourse import bacc, tile, mybir

    f32 = mybir.dt.float32
    bf16 = mybir.dt.bfloat16
    nc = bacc.Bacc(
        "TRN2", target_bir_lowering=False, debug=False, num_devices=NCORES
    )

    # partition-major pack: chunk L at rows [L*128, (L+1)*128), row p =
    # partition p holding [stripe, k, 512] contiguously (8KB/partition)
    xT_d = nc.dram_tensor(
        "xT4", [NLOC * 128, HALF * KCH * STRIPE], bf16, kind="ExternalInput"
    )
    cb_d = nc.dram_tensor("cb", [128, CB_BF_W], bf16, kind="ExternalInput")
    cf_d = nc.dram_tensor("cf", [128, CF_W], f32, kind="ExternalInput")
    # five stage regions side by side: [1024, 1+1+2+2+2 chunks] bf16
    out_d = nc.dram_tensor("out", [CHUNK, 8 * CHUNK], bf16, kind="ExternalOutput")

    with tile.TileContext(nc) as tc:
        with (
            tc.tile_pool(name="const", bufs=1) as cpool,
            tc.tile_pool(name="xkp", bufs=4) as xkpool,
            tc.tile_pool(name="work", bufs=4) as wpool,
            tc.tile_pool(name="big", bufs=8) as bigpool,
            tc.tile_pool(name="psum", bufs=4, space="PSUM") as pspool,
            tc.tile_pool(name="pspc", bufs=2, space="PSUM") as pspc,
            tc.tile_pool(name="psbc", bufs=2, space="PSUM") as psbc,
        ):
            # ---- constants: two packed blobs, two dispatches ----
            cb_sb = cpool.tile([128, CB_BF_W], bf16)
            nc.sync.dma_start(cb_sb[:], cb_d.ap())
            cf_sb = cpool.tile([128, CF_W], f32)
            nc.sync.dma_start(cf_sb[:], cf_d.ap())

            w1_sb = cb_sb[:, CB_W1 : CB_W1 + KCH * H].rearrange(
                "p (k h) -> p k h", k=KCH
            )
            w2b_sb = cb_sb[0:H, CB_W2B : CB_W2B + 128]
            w2_sb = cb_sb[0:H, CB_W2 : CB_W2 + 1]
            b1_sb = cf_sb[0:H, CF_B1 : CF_B1 + 1]
            b2b_sb = cf_sb[:, CF_B2 : CF_B2 + 1]

            # column proj (+b2) for all 4 chunks, position order COL_ORDER
            bcolall = cpool.tile([128, NLOC * CHUNK], bf16)
            # per-partition proj scalars ([128, CHUNK//128] per local chunk)
            # (must stay f32: tensor_scalar requires an f32 scalar operand)
            projcol = [
                cpool.tile([128, CHUNK // 128], f32, name=f"projcol{i}")
                for i in range(NLOC)
            ]

            xks = {}
            for loc in COMPUTE_ORDER:
                xk = xkpool.tile(
                    [128, HALF, KCH * STRIPE], bf16, tag="xk", name=f"xk{loc}"
                )
                nc.sync.dma_start(
                    xk[:],
                    xT_d.ap()[loc * 128 : (loc + 1) * 128, :].rearrange(
                        "p (s k) -> p s k", s=HALF
                    ),
                )
                xks[loc] = xk

            emit_ctr = [0]

            def emit_band(p, c0, w, d0, groups=range(CHUNK // 128)):
                # adds all on DVE (16-bit 2x mode); store dispatches
                # alternate between the two HW-DGE queues (sync + ACT) so
                # descriptor generation is not the store-phase bottleneck
                for g in groups:
                    ot = bigpool.tile(
                        [128, w], bf16, tag="ot", name=f"ot{d0}_{g}"
                    )
                    nc.vector.tensor_scalar_add(
                        ot[:], bcolall[:, c0 : c0 + w], projcol[p][:, g : g + 1]
                    )
                    r0 = g * 128
                    # first tiles go on the ACT queue only: the sync queue
                    # still has pending x reads in front of it
                    k = emit_ctr[0]
                    emit_ctr[0] += 1
                    eng = nc.scalar if (k < 8 or k % 2 == 0) else nc.sync
                    eng.dma_start(out_d.ap()[r0 : r0 + 128, d0 : d0 + w], ot[:])

            done = set()
            emitted = set()
            for loc in COMPUTE_ORDER:
                xk = xks[loc]
                pc_ps = None
                if loc != 2:
                    # batched proj scalars: 4 one-column matmuls per stripe
                    # into one PSUM tile, one copy per stripe
                    pc_ps = pspc.tile([128, CHUNK // 128], f32, tag="pc")
                for half in range(HALF):
                    seqT_ps = pspool.tile([H, STRIPE], f32, tag="seqT")
                    for k in range(KCH):
                        nc.tensor.matmul(
                            seqT_ps[:],
                            w1_sb[:, k, :],
                            xk[:, half, k * STRIPE : (k + 1) * STRIPE],
                            start=(k == 0),
                            stop=(k == KCH - 1),
                        )
                    seqT_sb = wpool.tile([H, STRIPE], bf16, tag="seqT_sb")
                    # relu(x + b1) as a fused DVE op
                    nc.vector.tensor_scalar(
                        seqT_sb[:],
                        seqT_ps[:],
                        b1_sb,
                        0.0,
                        op0=mybir.AluOpType.add,
                        op1=mybir.AluOpType.max,
                    )
                    # broadcast proj of this stripe across all 128 partitions
                    # in one matmul, folding b2 into the psum->sbuf copy
                    bc_ps = psbc.tile([128, STRIPE], f32, tag="bc")
                    nc.tensor.matmul(bc_ps[:], w2b_sb, seqT_sb[:])
                    base = COL_BASE[loc]
                    nc.vector.tensor_scalar_add(
                        bcolall[:, base + half * STRIPE : base + (half + 1) * STRIPE],
                        bc_ps[:],
                        b2b_sb,
                    )
                    # position 2 never appears as a band row; skip its scalars
                    if loc != 2:
                        npc = STRIPE // 128
                        for c in range(npc):
                            col = half * npc + c
                            nc.tensor.matmul(
                                pc_ps[:, col : col + 1],
                                seqT_sb[:, c * 128 : (c + 1) * 128],
                                w2_sb,
                            )
                        nc.vector.tensor_copy(
                            projcol[loc][:, half * npc : (half + 1) * npc],
                            pc_ps[:, half * npc : (half + 1) * npc],
                        )
                    if loc == COMPUTE_ORDER[0] and half == 0:
                        # the top-left [512,512] of the diag block depends
                        # only on this first stripe: emit it now so the
                        # first stores dispatch ~4us earlier
                        emit_band(0, 0, STRIPE, 0, groups=range(4))

                done.add(loc)
                for bi, (p, c0, w, d0) in enumerate(BANDS):
                    need = {p} | {
                        q for q in range(NLOC) if c0 <= COL_BASE[q] < c0 + w
                    }
                    if bi not in emitted and need <= done:
                        emitted.add(bi)
                        if bi == 0:
                            # rest of the diag block around the early piece
                            emit_band(0, STRIPE, STRIPE, STRIPE, groups=range(4))
                            emit_band(0, 0, CHUNK, 0, groups=range(4, 8))
                        else:
                            emit_band(p, c0, w, d0)

            for bi, (p, c0, w, d0) in enumerate(BANDS):
                if bi not in emitted:
                    emitted.add(bi)
                    emit_band(p, c0, w, d0)

    nc.compile()
    return nc


def kernel(gathered_sequences, W1, b1, w2, b2):
    global LAST_RESULTS
    from concourse import bass_utils

    if "nc" not in _state:
        _state["nc"] = _build()
    nc = _state["nc"]

    bf = ml_dtypes.bfloat16
    x = np.ascontiguousarray(gathered_sequences, dtype=np.float32)
    xT = np.ascontiguousarray(x.T)  # [D, N]

    # const blobs
    cb = np.zeros((128, CB_BF_W), dtype=bf)
    W1b = np.asarray(W1, dtype=np.float32).astype(bf)  # [D, H]
    # w1 field: [p, k*H + h] = W1[k*128 + p, h]
    cb[:, CB_W1 : CB_W1 + KCH * H] = (
        W1b.reshape(KCH, 128, H).transpose(1, 0, 2).reshape(128, KCH * H)
    )
    w2c = np.reshape(w2, (H, 1)).astype(np.float32).astype(bf)
    cb[0:H, CB_W2B : CB_W2B + 128] = np.repeat(w2c, 128, axis=1)
    cb[0:H, CB_W2 : CB_W2 + 1] = w2c
    cf = np.zeros((128, CF_W), dtype=np.float32)
    cf[0:H, CF_B1] = np.reshape(b1, (H,)).astype(np.float32)
    cf[:, CF_B2] = float(np.reshape(b2, ()))

    in_maps = []
    for m in range(NCORES):
        locs = [(m + a) % NCORES for a in LOCAL_OFFS]
        xT4 = np.concatenate(
            [xT[:, L * CHUNK : (L + 1) * CHUNK] for L in locs], axis=1
        )  # [D, LROWS]
        # partition-major pack: [NLOC*128, HALF*KCH*STRIPE] where row
        # L*128+p holds chunk L's [stripe, k, 512] block for partition p
        xT4p = np.ascontiguousarray(
            xT4.reshape(KCH, 128, NLOC, HALF, STRIPE)
            .transpose(2, 1, 3, 0, 4)
            .reshape(NLOC * 128, HALF * KCH * STRIPE)
            .astype(bf)
        )
        in_maps.append({"xT4": xT4p, "cb": cb, "cf": cf})

    res = bass_utils.run_bass_kernel_spmd(nc, in_maps, core_ids=list(range(NCORES)))
    LAST_RESULTS = res

    out = np.empty((N, N), dtype=np.float32)
    for m in range(NCORES):
        locs = [(m + a) % NCORES for a in LOCAL_OFFS]
        bands = np.asarray(res.results[m]["out"]).astype(np.float32)
        for p, c0, w, d0 in BANDS:
            gr = locs[p]
            cols = [
                COL_ORDER[i] for i in range(NLOC) if c0 <= i * CHUNK < c0 + w
            ]
            for j, q in enumerate(cols):
                gc = locs[q]
                out[
                    gr * CHUNK : (gr + 1) * CHUNK, gc * CHUNK : (gc + 1) * CHUNK
                ] = bands[:, d0 + j * CHUNK : d0 + (j + 1) * CHUNK]
    return out
